# revision 24
# baseline (speedup 1.0000x reference)
"""Trainium2 Bass kernel for nn_DiscriminationLoss (segment_reduce).

Strategy (8 NeuronCores, pixel-sharded):
  - Each core gets 1/8 of the 4M pixels: pred slice [8, 524288] f32 and
    labels slice [524288] i32.
  - Pixels are tiled [128 partitions x F free]. For each free column t
    (a "block" of 128 pixels), a one-hot matrix oh[p, j] = (labels[p,t]
    == j+1), j in 0..31 is built on DVE (label 0 = background dropped).
  - One-hot generation uses per-label tensor_scalar(is_equal) ops: a
    single-source op with dense step-1 16-bit APs engages the DVE
    4x_2p perf mode (tensor_tensor caps at 2x_1p); measured ~3.86
    elem/cycle/lane at 900-col chunks. This ~47us stream is the body's
    pacer, alongside the ~47-50us HBM stream of pred.
  - The pixel<->(partition, column) mapping is chunk-local:
    pixel = 128*coff + p*fcc + f for chunk [coff, coff+fcc). Labels
    are cast-DMA'd (int32->bf16, SWDGE) into per-chunk resident tiles
    at kernel start. The two small qb=4 chunks run first: their label
    tiles land earliest so the DVE stream starts ~5us sooner.
  - pred is scaled by 2^14 and cast to bf16 on ScalarE (the scale
    rides the activation's free affine; a second activation with
    scale=0, bias=1 writes the ones column used for counts).
  - PE uses 2-way COLUMN TILING: per matmul pair, two matmuls run
    concurrently in disjoint 64-column strips of the 128x128 array
    (tile_position (0,0) / (0,64), inferred from the PSUM slice base
    partition):
      psumA[54, 192] += stA[128, 54]^T @ ohA[128, 192]   (cols 0..63)
      psumB[54, 192] += stB[128, 54]^T @ ohB[128, 192]   (cols 64..127)
    st packs 6 blocks' [8 bf16 channels | ones]; oh packs 6 blocks'
    one-hots; only the diagonal [9, 32] sub-blocks per matmul are
    meaningful (host extracts them). Col-tiling roughly halves PE
    streaming time, so the PE is no longer the pacer. The two qb=4
    remainder chunks accumulate into separate PSUM column ranges
    (192.. and 320..) so each accumulation group is self-contained
    and bank `has_written` clears cannot corrupt other regions.
  - A short warmup burst of matmuls trips the PE HAM clock gate to
    2.4 GHz before the real stream arrives.
  - Each core emits [128, 512] (PSUM readout + warmup dump). Host sums
    partials over cores (the "psum" step of the sharding hint) and
    evaluates the tiny O(K^2) pairwise tail in f64.
"""

import sys
import functools

sys.path.insert(0, "/opt/trn_rl_repo")

import numpy as np

C = 8
K = 32
NCORES = 8
H = W = 2048
PTOT = H * W
PCORE = PTOT // NCORES  # 524288
SIGMA_DIS = 3.0
PRED_SCALE = float(2.0**14)

WARM_MMS = 16  # PE warmup matmuls (trip the HAM clock gate to 2.4 GHz)

# (chunk_cols, qb, psum_colbase). qb=6 chunks share psum cols 0..191
# via one accumulation group per array half; each qb=4 chunk gets its
# own psum col range (+ its own group). The small qb=4 chunks run
# first: their label tiles land earliest so the DVE stream starts
# ~5us sooner, and they cover the window while the 900-col label
# tiles are still arriving.
CHUNKS = [(64, 4, 192), (900, 6, 0), (900, 6, 0), (900, 6, 0), (900, 6, 0),
          (432, 4, 320)]
GSPLIT = {900: [450, 450], 432: [216, 216], 64: [64]}


def build_nc(pcore=PCORE, warm=WARM_MMS):
    import concourse.bacc as bacc
    import concourse.tile as tile
    import concourse.mybir as mybir
    from contextlib import ExitStack

    ftot = pcore // 128
    assert sum(c for c, _, _ in CHUNKS) == ftot
    f32 = mybir.dt.float32
    bf16 = mybir.dt.bfloat16
    i32 = mybir.dt.int32

    nch = C + 1
    ones_col = C
    fgmax = 450
    fcmax = max(c for c, _, _ in CHUNKS)

    nc = bacc.Bacc(
        "TRN2", target_bir_lowering=False, debug=False, num_devices=NCORES
    )
    pred_ext = nc.dram_tensor("pred", [C, pcore], f32, kind="ExternalInput")
    lab_ext = nc.dram_tensor("labels", [pcore], i32, kind="ExternalInput")
    out_ext = nc.dram_tensor("out_s", [128, 512], f32, kind="ExternalOutput")

    with tile.TileContext(nc) as tc, ExitStack() as ctx:
        const_pool = ctx.enter_context(tc.tile_pool(name="const", bufs=1))
        lab_pool = ctx.enter_context(tc.tile_pool(name="lab", bufs=1))
        slab32_pool = ctx.enter_context(tc.tile_pool(name="slab32", bufs=2))
        slabh_pool = ctx.enter_context(tc.tile_pool(name="slabh", bufs=3))
        oh_pool = ctx.enter_context(tc.tile_pool(name="oh", bufs=2))
        psum_pool = ctx.enter_context(tc.tile_pool(name="psum", bufs=1, space="PSUM"))
        out_pool = ctx.enter_context(tc.tile_pool(name="outp", bufs=1))

        # warm tile: memset early (no input deps) — feeds PE warmup and
        # the ones-column activations
        warm_t = const_pool.tile([128, 256], bf16)
        nc.gpsimd.memset(warm_t[:], 1.0)

        # all labels resident up front, one bf16 tile per chunk with the
        # chunk-local pixel mapping; SWDGE cast-DMA int32 -> bf16 keeps
        # the ScalarE queue (pred casts) and DVE (one-hot stream) clear.
        lab_ts = []
        coff = 0
        for ci, (fcc, _, _) in enumerate(CHUNKS):
            lt = lab_pool.tile([128, fcc], bf16, tag=f"lab{ci}")
            nc.gpsimd.dma_start(
                lt[:],
                lab_ext[128 * coff : 128 * (coff + fcc)].rearrange(
                    "(p f) -> p f", p=128
                ),
            )
            lab_ts.append(lt)
            coff += fcc

        psum_full = psum_pool.tile([128, 512], f32)

        # PE warmup: dense matmuls so the HAM clock gate opens before
        # the real matmul stream arrives.
        warm_ps = psum_pool.tile([128, 256], f32)
        if warm:
            for w in range(warm):
                nc.tensor.matmul(
                    warm_ps[:],
                    warm_t[:, :128],
                    warm_t[:, :256],
                    start=(w == 0),
                    stop=(w == warm - 1),
                )

        # accumulation bookkeeping: the qb=6 chunks form one group (per
        # array half) across all four chunks; each qb=4 chunk is its own
        # group. First/last matmul per half carries start/stop.
        nmm6 = sum(fcc // qb for fcc, qb, _ in CHUNKS if qb == 6)
        seen6 = 0

        coff = 0
        for ci, (fcc, qb, colbase) in enumerate(CHUNKS):
            stw = nch * qb          # stationary cols per matmul
            mvw = K * qb            # moving cols per matmul
            pred_chunk = pred_ext[:, 128 * coff : 128 * (coff + fcc)].rearrange(
                "c (p f) -> p c f", p=128
            )
            # ONE pred DMA per chunk (fewer, larger HBM transfers —
            # per-transfer completion bubbles cost ~0.6us each); casts
            # stay at half-chunk granularity for pipelining.
            slab32 = slab32_pool.tile([128, C * fcmax], f32, tag="slab32")
            s32c = slab32[:, : C * fcc]
            nc.sync.dma_start(
                s32c.rearrange("p (c f) -> p c f", c=C),
                pred_chunk[:],
            )
            s32c_r = s32c.rearrange("p (c tg b) -> p tg c b", c=C, b=qb)
            slabhs = []
            goff = 0
            for fgg in GSPLIT[fcc]:
                # slabh layout: [p, (tg, c, b)] — each tg's stationary
                # [128, nch*qb] is a contiguous slice.
                slabh = slabh_pool.tile([128, nch * fgmax], bf16, tag="slabh")
                slabh_r = slabh[:, : nch * fgg].rearrange(
                    "p (tg c b) -> p tg c b", c=nch, b=qb
                )
                slab32_r = s32c_r[:, goff // qb : (goff + fgg) // qb, :, :]
                # scaled bf16 cast on ScalarE: out = Copy(in * 2^14)
                nc.scalar.activation(
                    slabh_r[:, :, :C, :],
                    slab32_r,
                    mybir.ActivationFunctionType.Copy,
                    scale=PRED_SCALE,
                )
                # ones column via ACT: Copy(0*x + 1) = 1.0; input warm_t
                # (always ready) so this op can run before the pred DMA.
                nc.scalar.activation(
                    slabh_r[:, :, ones_col, :],
                    warm_t[:, :1].unsqueeze(2).broadcast_to([128, fgg // qb, qb]),
                    mybir.ActivationFunctionType.Copy,
                    bias=1.0,
                    scale=0.0,
                )
                slabhs.append((goff, fgg, slabh))
                goff += fgg

            # one-hot chunk: per-label tensor_scalar(is_equal) at DVE 4x.
            # oh layout: [p, (tg, j, b)] — each tg's moving operand
            # [128, K*qb] is a contiguous slice.
            oh = oh_pool.tile([128, K * fcmax], bf16, tag="oh")
            oh_r = oh[:, : K * fcc].rearrange(
                "p (tg j b) -> p tg j b", j=K, b=qb
            )  # [128, fcc/qb, K, qb]
            lab_in = lab_ts[ci][:].rearrange("p (tg b) -> p tg b", b=qb)
            for j in range(K):
                nc.vector.tensor_scalar(
                    oh_r[:, :, j, :],
                    lab_in,
                    float(j + 1),
                    None,
                    mybir.AluOpType.is_equal,
                )
            # col-tiled matmul pairs: tg even -> array cols 0..63
            # (psum partitions 0..stw-1), tg odd -> cols 64..127
            ntg_c = fcc // qb
            for goff, fgg, slabh in slabhs:
                for tgl in range(fgg // qb):
                    tg = goff // qb + tgl  # chunk-local tg
                    half = tg % 2
                    pbase = 64 * half
                    if qb == 6:
                        start = seen6 in (0, 1)
                        stop = seen6 in (nmm6 - 2, nmm6 - 1)
                        seen6 += 1
                    else:
                        start = tg in (0, 1)
                        stop = tg in (ntg_c - 2, ntg_c - 1)
                    nc.tensor.matmul(
                        psum_full[pbase : pbase + stw, colbase : colbase + mvw],
                        slabh[:, tgl * stw : (tgl + 1) * stw],
                        oh[:, tg * mvw : (tg + 1) * mvw],
                        start=start,
                        stop=stop,
                        skip_group_check=True,
                    )
            # keep-warm dummies: run right after this chunk's burst and
            # shorten the PE idle gap below the ~3.4us HAM re-throttle
            # window (cold bursts cascade into DVE oh-buffer stalls).
            if ci < len(CHUNKS) - 1:
                for w in range(8):
                    nc.tensor.matmul(
                        warm_ps[:],
                        warm_t[:, :128],
                        warm_t[:, :256],
                        start=(w == 0),
                        stop=(w == 7),
                    )
            coff += fcc

        outt = out_pool.tile([128, 512], f32)
        nc.vector.memset(outt[:], 0.0)
        nc.vector.tensor_copy(outt[:118, :448], psum_full[:118, :448])
        if warm:
            nc.vector.tensor_copy(outt[96:97, 448:512], warm_ps[96:97, :64])
        nc.sync.dma_start(out_ext[:], outt[:])
    nc.compile()
    return nc


@functools.lru_cache(maxsize=1)
def _get_program():
    return build_nc()


def make_in_maps(pred_flat, labels_flat):
    in_maps = []
    for i in range(NCORES):
        sl = slice(i * PCORE, (i + 1) * PCORE)
        in_maps.append(
            {
                "pred": np.ascontiguousarray(pred_flat[:, sl]),
                "labels": np.ascontiguousarray(labels_flat[sl]),
            }
        )
    return in_maps


def finish_host(parts, num_kernel):
    """parts: per-core [128, 512] partials. Tiny O(K^2) tail in f64."""
    nch = C + 1
    total = np.sum([p.astype(np.float64) for p in parts], axis=0)
    acc = np.zeros((nch, K))
    for qb, colbase in ((6, 0), (4, 192), (4, 320)):
        for pbase in (0, 64):
            r = total[pbase : pbase + nch * qb, colbase : colbase + K * qb]
            r = r.reshape(nch, qb, K, qb)
            acc += r[:, np.arange(qb), :, np.arange(qb)].sum(axis=0)
    S = acc[:C, :] / PRED_SCALE  # [8, 32]
    N = acc[C, :]  # [32]
    A = N * np.sum(S * S, axis=0)  # [32]
    kk = int(num_kernel)
    A = A[:kk]
    pair = A[:, None] + A[None, :]
    Dm = np.maximum(SIGMA_DIS - np.sqrt(pair), 0.0)
    term = np.log(Dm * Dm + 1.0)
    L = float(np.sum(np.triu(term, k=1)))
    L *= (kk - 1) / kk
    return np.float32(L)


_last_results = None


def kernel(pred_similarities, regions_mask, kernel_labels, num_kernel, **kw):
    global _last_results
    from concourse.bass_utils import run_bass_kernel_spmd

    pred_flat = np.asarray(pred_similarities, dtype=np.float32).reshape(C, PTOT)
    labels_flat = np.asarray(kernel_labels, dtype=np.int32).reshape(PTOT)

    nc = _get_program()
    in_maps = make_in_maps(pred_flat, labels_flat)
    res = run_bass_kernel_spmd(nc, in_maps, list(range(NCORES)))
    _last_results = res
    parts = [res.results[i]["out_s"] for i in range(NCORES)]
    return finish_host(parts, num_kernel)
